# revision 29
# baseline (speedup 1.0000x reference)
# Depthwise 4x4 conv (DiagonalwiseRefactorization) on 8 TRN2 NeuronCores.
#
# The mask zeroes every weight except weight[c, c % 64], and with
# feature_group_count=8 the grouped conv collapses to a depthwise conv:
#   out[n, c, ho, wo] = sum_{kh, kw} w[c, kh, kw] * xpad[n, c, ho+kh, wo+kw]
# with pad=1, stride=1: (16, 512, 64, 64) -> (16, 512, 63, 63).
#
# Device strategy (per core: 64 channels x 16 images, no inter-core comm):
#   For each width-tap kw, the H-direction conv is a banded-Toeplitz matmul:
#     out[c, :, wo] += T_c_kw.T @ xrow[c, :, wo+kw]
#   where T_c_kw[h, ho] = w[c, h-ho+1, kw] (64x64, 4 diagonals; the H padding
#   falls out of the band clipping). Two channels run concurrently in opposite
#   32x32-quadrant groups of the PE array (tile_position (0,0) / (64,64));
#   the 4 kw taps accumulate in PSUM. bf16 matmuls, fp32 PSUM accumulate,
#   bf16 store (un-swizzled + upcast on host).
#
# Host does layout only: bf16 cast + swizzle of x to [pair, c'*64+h, n, wpad],
# building the banded lhsT blocks from (weight*mask), and un-permuting the
# swizzled output.

import sys
import types

import numpy as np
import ml_dtypes

BF16 = ml_dtypes.bfloat16

N_CORES = 8
IMGS = 16
CH_TOT = 512
CH = CH_TOT // N_CORES  # 64 channels per core
PAIRS = CH // 2  # 32
H = W = 64
HO = WO = 63
WPAD = W + 2  # 66 (one zero column each side)
NHALF = IMGS // 2  # 8 images per psum tile
NFREE = NHALF * WO  # 504 <= 512 (one PSUM bank)


def _install_axon_hooks_shim():
    """Make trace=True work under axon: bass_utils imports
    antenv.axon_hooks, which the container's antenv stub lacks."""
    try:
        import antenv.axon_hooks  # noqa: F401

        return
    except ImportError:
        pass
    try:
        import antenv
    except ImportError:
        return
    mod = types.ModuleType("antenv.axon_hooks")
    mod._hook = None

    def set_axon_ntff_profile_hook(h):
        mod._hook = h

    def get_axon_ntff_profile_hook():
        return mod._hook

    mod.set_axon_ntff_profile_hook = set_axon_ntff_profile_hook
    mod.get_axon_ntff_profile_hook = get_axon_ntff_profile_hook
    sys.modules["antenv.axon_hooks"] = mod
    antenv.axon_hooks = mod
    try:
        from trn_agent_boot.trn_boot import _ntff_profile_via_ctypes

        hook = _ntff_profile_via_ctypes("/opt/axon/libaxon_pjrt.so")
        if hook is not None:
            mod._hook = hook
    except Exception:
        pass


_install_axon_hooks_shim()

import concourse.bacc as bacc  # noqa: E402
import concourse.mybir as mybir  # noqa: E402
import concourse.tile as tile  # noqa: E402
from concourse.bass_utils import run_bass_kernel_spmd  # noqa: E402

LAST_RESULT = None
_NC_CACHE = None


XCOLS = IMGS * W  # 1024 (no pad columns; kw edges handled by clipped matmuls)
WCOLS = 4 * H  # 256
INCOLS = XCOLS + WCOLS  # 1280: x and lhsT packed into one DMA per pair

# Per width-tap kw: x column range [xc0, xc1) and output wo range [wo0, wo1).
# out[wo] += w[.., kw] * x[wo + kw - 1]; clipped where x would be padding.
# kw=1 goes first: it covers the full wo range, so its start=True write sets
# PSUM has_written everywhere before the partial-range taps accumulate.
KW_PLAN = [
    (1, 0, 63, 0, 63),  # kw, xc0, xc1, wo0, wo1
    (2, 1, 64, 0, 63),
    (0, 0, 62, 1, 63),
    (3, 2, 64, 0, 62),
]


def _build_nc():
    # Bass.__init__ emits four [128,1] const-AP memsets on GpSimd whose DMA
    # completion delays the first all-engine barrier; this kernel never reads
    # the const APs (matmul/copy/dma only), so skip those preamble memsets.
    import concourse.bass as bassmod

    orig_memset = bassmod.BassGpSimd.memset
    bassmod.BassGpSimd.memset = lambda self, ap, constant: None
    try:
        nc = bacc.Bacc(
            "TRN2", target_bir_lowering=False, debug=False, num_devices=N_CORES
        )
    finally:
        bassmod.BassGpSimd.memset = orig_memset

    xd = nc.dram_tensor(
        "xin", [PAIRS, 128, INCOLS], mybir.dt.bfloat16, kind="ExternalInput"
    )
    od = nc.dram_tensor(
        "out", [PAIRS, 128, 2, NFREE], mybir.dt.bfloat16, kind="ExternalOutput"
    )

    with tile.TileContext(nc) as tc:
        with (
            tc.tile_pool(name="xp", bufs=8) as xp,
            tc.tile_pool(name="ps", bufs=4, space="PSUM") as ps,
            tc.tile_pool(name="op", bufs=8) as op,
        ):
            # Warm up the PE HAM clock gate (1.2 -> 2.4 GHz needs ~3.4 us of
            # sustained matmul activity) inside the first x-DMA's shadow, so
            # the real matmuls start at full clock.
            wsrc = op.tile([128, 128], mybir.dt.bfloat16, name="warmsrc")
            nc.vector.memset(wsrc[:], 0.0)
            warm = ps.tile([128, NFREE], mybir.dt.float32, name="pt0")
            for _ in range(27):
                nc.tensor.matmul(
                    warm[0:64, 0:128],
                    lhsT=wsrc[:, 0:64],
                    rhs=wsrc[:],
                    start=True,
                    stop=True,
                )

            for pair in range(PAIRS):
                xt = xp.tile([128, INCOLS], mybir.dt.bfloat16)
                if pair == 0:
                    # Split the first load so image-half 0 (+ weights) lands
                    # sooner and the real matmuls start earlier.
                    nc.sync.dma_start(
                        out=xt[:, 0 : NHALF * W], in_=xd[pair, :, 0 : NHALF * W]
                    )
                    nc.sync.dma_start(
                        out=xt[:, XCOLS:INCOLS], in_=xd[pair, :, XCOLS:INCOLS]
                    )
                    nc.sync.dma_start(
                        out=xt[:, NHALF * W : XCOLS],
                        in_=xd[pair, :, NHALF * W : XCOLS],
                    )
                else:
                    # Pairs 1-4 load via the Scalar HWDGE ring (idle until its
                    # first copy ~12.5 us) so the prefetch ramp fills two
                    # dispatch queues in parallel and the early-stream
                    # input-wait stalls disappear.
                    ine = nc.scalar if pair <= 4 else nc.sync
                    ine.dma_start(out=xt[:], in_=xd[pair])
                xv = xt[:, 0:XCOLS].rearrange("p (n w) -> p n w", w=W)
                wv = xt[:, XCOLS:INCOLS].rearrange("p (k m) -> p k m", m=H)

                pts = [
                    ps.tile([128, NHALF, WO], mybir.dt.float32, name=f"pt{h}")
                    for h in range(2)
                ]
                for kw, xc0, xc1, wo0, wo1 in KW_PLAN:
                    for half in range(2):
                        rhs = xv[:, half * NHALF : (half + 1) * NHALF, xc0:xc1]
                        # Channel A in PE quadrants (rows 0-63, cols 0-63),
                        # channel B in (rows 64-127, cols 64-127): the two
                        # matmuls run concurrently in disjoint subarrays.
                        # NOTE: kw-outer order is load-bearing — consecutive
                        # slots share lhsT, which is what lets the PE stream
                        # at ~215 ns/slot (half-serial order measured 86 us).
                        nc.tensor.matmul(
                            pts[half][0:64, :, wo0:wo1],
                            lhsT=wv[0:64, kw, :],
                            rhs=rhs[0:64],
                            start=(kw == 1),
                            stop=(kw == 3),
                            tile_position=(0, 0),
                        )
                        nc.tensor.matmul(
                            pts[half][64:128, :, wo0:wo1],
                            lhsT=wv[64:128, kw, :],
                            rhs=rhs[64:128],
                            start=(kw == 1),
                            stop=(kw == 3),
                            tile_position=(64, 64),
                        )
                ot = op.tile([128, 2, NFREE], mybir.dt.bfloat16)
                if pair == PAIRS - 1:
                    # Tail: split each copy across ACT and DVE and store each
                    # half as soon as its copy lands.
                    for half in range(2):
                        nc.scalar.copy(
                            ot[:, half, 0 : NFREE // 2], pts[half][:, 0:4, :]
                        )
                        nc.vector.tensor_copy(
                            ot[:, half, NFREE // 2 : NFREE], pts[half][:, 4:8, :]
                        )
                        # Sync's HWDGE ring is idle by the tail (inputs done);
                        # dispatch the two half-stores on different rings so
                        # they overlap instead of serializing on Scalar.
                        eng = nc.scalar if half == 0 else nc.sync
                        eng.dma_start(out=od[pair, :, half], in_=ot[:, half, :])
                else:
                    nc.scalar.copy(ot[:, 0, :], pts[0][:])
                    nc.vector.tensor_copy(ot[:, 1, :], pts[1][:])
                    nc.scalar.dma_start(out=od[pair], in_=ot[:])
    nc.compile()
    return nc


def _get_nc():
    global _NC_CACHE
    if _NC_CACHE is None:
        _NC_CACHE = _build_nc()
    return _NC_CACHE


def _prep_x(x):
    """x (16, 512, 64, 64) f32 -> per-core (PAIRS, 128, XCOLS) bf16.

    Partition index p = c'*64 + h for channel pair slot c' in {0, 1};
    free layout [n, w] (no pad columns - kw edges use clipped matmul ranges).
    """
    maps = []
    for k in range(N_CORES):
        xc = x[:, k * CH : (k + 1) * CH]  # (16, 64, 64, 64)
        t = xc.transpose(1, 2, 0, 3)  # (ch, h, n, w)
        maps.append(t.astype(BF16).reshape(PAIRS, 128, XCOLS))
    return maps


def _prep_w(wc):
    """wc (512, 4, 4) f32 masked per-channel weights ->
    per-core (PAIRS, 128, WCOLS) bf16 banded lhsT blocks.

    lhsT[pair, c'*64 + h, kw*H + ho] = wc[ch, h - ho + 1, kw]
    for 0 <= h - ho + 1 <= 3, ho <= 62 (column 63 stays zero).
    """
    maps = []
    for k in range(N_CORES):
        wk = wc[k * CH : (k + 1) * CH]  # (64, 4, 4) [ch, kh, kw]
        blocks = np.zeros((CH, 4, H, H), dtype=np.float32)  # [ch, kw, h, ho]
        ho = np.arange(HO)
        for kh in range(4):
            h = ho + kh - 1
            v = (h >= 0) & (h < H)
            blocks[:, :, h[v], ho[v]] = wk[:, kh, :][:, :, None]
        # [ch, kw, h, ho] -> [pair, c'*64+h, kw*H + ho]
        lt = blocks.transpose(0, 2, 1, 3).reshape(PAIRS, 128, WCOLS)
        maps.append(lt.astype(BF16))
    return maps


def _prep_in(x, wc):
    xs = _prep_x(x)
    ws = _prep_w(wc)
    return [
        np.ascontiguousarray(np.concatenate([xs[k], ws[k]], axis=2))
        for k in range(N_CORES)
    ]


def _unswizzle(out_dev):
    """(PAIRS, 128, 2, NFREE) bf16 -> (16, 64, 63, 63) f32 for one core."""
    r = out_dev.reshape(PAIRS, 2, H, 2, NHALF, WO)  # [pair, c', ho64, half, n', wo]
    t = r.transpose(3, 4, 0, 1, 2, 5)  # [half, n', pair, c', ho64, wo]
    return np.ascontiguousarray(
        t.reshape(IMGS, CH, H, WO)[:, :, :HO, :].astype(np.float32)
    )


def kernel(x, weight, mask, groups=8, stride=1, _trace=False, _trace_kwargs=None):
    global LAST_RESULT
    x = np.ascontiguousarray(np.asarray(x, dtype=np.float32))
    weight = np.asarray(weight, dtype=np.float32)
    mask = np.asarray(mask, dtype=np.float32)

    # Masked weights collapse to one 4x4 filter per output channel.
    wc = (weight * mask).sum(axis=1)  # (512, 4, 4)

    ins = _prep_in(x, wc)
    in_maps = [{"xin": ins[k]} for k in range(N_CORES)]

    nc = _get_nc()
    kwargs = {}
    if _trace:
        kwargs["trace"] = True
        if _trace_kwargs:
            kwargs.update(_trace_kwargs)
    res = run_bass_kernel_spmd(nc, in_maps, core_ids=list(range(N_CORES)), **kwargs)
    LAST_RESULT = res

    outs = [_unswizzle(res.results[k]["out"]) for k in range(N_CORES)]
    return np.concatenate(outs, axis=1)


def emulate(x, weight, mask, groups=8, stride=1):
    """Pure-numpy emulation of the device math (same bf16 rounding and
    packing) - validates host prep + Toeplitz construction without HW."""
    x = np.asarray(x, dtype=np.float32)
    wc = (np.asarray(weight, np.float32) * np.asarray(mask, np.float32)).sum(axis=1)
    ins = _prep_in(x, wc)
    outs = []
    for k in range(N_CORES):
        out_dev = np.zeros((PAIRS, 128, 2, NFREE), dtype=BF16)
        for pair in range(PAIRS):
            xin = ins[k][pair, :, 0:XCOLS].astype(np.float32)
            xin = xin.reshape(128, IMGS, W)
            wt = ins[k][pair, :, XCOLS:INCOLS].astype(np.float32)
            wt = wt.reshape(128, 4, H)
            for half in range(2):
                acc = np.zeros((128, NHALF, WO), dtype=np.float32)
                for kw, xc0, xc1, wo0, wo1 in KW_PLAN:
                    rhs = xin[:, half * NHALF : (half + 1) * NHALF, xc0:xc1]
                    acc[0:64, :, wo0:wo1] += np.einsum(
                        'km,knw->mnw', wt[0:64, kw, :], rhs[0:64])
                    acc[64:128, :, wo0:wo1] += np.einsum(
                        'km,knw->mnw', wt[64:128, kw, :], rhs[64:128])
                out_dev[pair, :, half, :] = acc.reshape(128, NFREE).astype(BF16)
        outs.append(_unswizzle(out_dev))
    return np.concatenate(outs, axis=1)


# revision 30
# speedup vs baseline: 1.0607x; 1.0607x over previous
# Depthwise 4x4 conv (DiagonalwiseRefactorization) on 8 TRN2 NeuronCores.
#
# The mask zeroes every weight except weight[c, c % 64], and with
# feature_group_count=8 the grouped conv collapses to a depthwise conv:
#   out[n, c, ho, wo] = sum_{kh, kw} w[c, kh, kw] * xpad[n, c, ho+kh, wo+kw]
# with pad=1, stride=1: (16, 512, 64, 64) -> (16, 512, 63, 63).
#
# Device strategy (per core: 64 channels x 16 images, no inter-core comm):
#   For each width-tap kw, the H-direction conv is a banded-Toeplitz matmul:
#     out[c, :, wo] += T_c_kw.T @ xrow[c, :, wo+kw]
#   where T_c_kw[h, ho] = w[c, h-ho+1, kw] (64x64, 4 diagonals; the H padding
#   falls out of the band clipping). Two channels run concurrently in opposite
#   32x32-quadrant groups of the PE array (tile_position (0,0) / (64,64));
#   the 4 kw taps accumulate in PSUM. bf16 matmuls, fp32 PSUM accumulate,
#   bf16 store (un-swizzled + upcast on host).
#
# Host does layout only: bf16 cast + swizzle of x to [pair, c'*64+h, n, wpad],
# building the banded lhsT blocks from (weight*mask), and un-permuting the
# swizzled output.

import sys
import types

import numpy as np
import ml_dtypes

BF16 = ml_dtypes.bfloat16

N_CORES = 8
IMGS = 16
CH_TOT = 512
CH = CH_TOT // N_CORES  # 64 channels per core
PAIRS = CH // 2  # 32
H = W = 64
HO = WO = 63
WPAD = W + 2  # 66 (one zero column each side)
NHALF = IMGS // 2  # 8 images per psum tile
NFREE = NHALF * WO  # 504 <= 512 (one PSUM bank)


def _install_axon_hooks_shim():
    """Make trace=True work under axon: bass_utils imports
    antenv.axon_hooks, which the container's antenv stub lacks."""
    try:
        import antenv.axon_hooks  # noqa: F401

        return
    except ImportError:
        pass
    try:
        import antenv
    except ImportError:
        return
    mod = types.ModuleType("antenv.axon_hooks")
    mod._hook = None

    def set_axon_ntff_profile_hook(h):
        mod._hook = h

    def get_axon_ntff_profile_hook():
        return mod._hook

    mod.set_axon_ntff_profile_hook = set_axon_ntff_profile_hook
    mod.get_axon_ntff_profile_hook = get_axon_ntff_profile_hook
    sys.modules["antenv.axon_hooks"] = mod
    antenv.axon_hooks = mod
    try:
        from trn_agent_boot.trn_boot import _ntff_profile_via_ctypes

        hook = _ntff_profile_via_ctypes("/opt/axon/libaxon_pjrt.so")
        if hook is not None:
            mod._hook = hook
    except Exception:
        pass


_install_axon_hooks_shim()

import concourse.bacc as bacc  # noqa: E402
import concourse.mybir as mybir  # noqa: E402
import concourse.tile as tile  # noqa: E402
from concourse.bass_utils import run_bass_kernel_spmd  # noqa: E402

LAST_RESULT = None
_NC_CACHE = None


XCOLS = IMGS * W  # 1024 (no pad columns; kw edges handled by clipped matmuls)
WCOLS = 4 * H  # 256
INCOLS = XCOLS + WCOLS  # 1280: x and lhsT packed into one DMA per pair

# Per width-tap kw: x column range [xc0, xc1) and output wo range [wo0, wo1).
# out[wo] += w[.., kw] * x[wo + kw - 1]; clipped where x would be padding.
# kw=1 goes first: it covers the full wo range, so its start=True write sets
# PSUM has_written everywhere before the partial-range taps accumulate.
KW_PLAN = [
    (1, 0, 63, 0, 63),  # kw, xc0, xc1, wo0, wo1
    (2, 1, 64, 0, 63),
    (0, 0, 62, 1, 63),
    (3, 2, 64, 0, 62),
]


def _build_nc():
    # Bass.__init__ emits four [128,1] const-AP memsets on GpSimd whose DMA
    # completion delays the first all-engine barrier; this kernel never reads
    # the const APs (matmul/copy/dma only), so skip those preamble memsets.
    import concourse.bass as bassmod

    orig_memset = bassmod.BassGpSimd.memset
    bassmod.BassGpSimd.memset = lambda self, ap, constant: None
    try:
        nc = bacc.Bacc(
            "TRN2", target_bir_lowering=False, debug=False, num_devices=N_CORES
        )
    finally:
        bassmod.BassGpSimd.memset = orig_memset

    xd = nc.dram_tensor(
        "xin", [PAIRS, 128, INCOLS], mybir.dt.bfloat16, kind="ExternalInput"
    )
    od = nc.dram_tensor(
        "out", [PAIRS, 128, 2, NFREE], mybir.dt.bfloat16, kind="ExternalOutput"
    )

    with tile.TileContext(nc) as tc:
        with (
            tc.tile_pool(name="xp", bufs=8) as xp,
            tc.tile_pool(name="ps", bufs=4, space="PSUM") as ps,
            tc.tile_pool(name="op", bufs=8) as op,
        ):
            # Warm up the PE HAM clock gate (1.2 -> 2.4 GHz needs ~3.4 us of
            # sustained matmul activity) inside the first x-DMA's shadow, so
            # the real matmuls start at full clock.
            wsrc = op.tile([128, 128], mybir.dt.bfloat16, name="warmsrc")
            nc.vector.memset(wsrc[:], 0.0)
            warm = ps.tile([128, NFREE], mybir.dt.float32, name="pt0")
            for _ in range(27):
                nc.tensor.matmul(
                    warm[0:64, 0:128],
                    lhsT=wsrc[:, 0:64],
                    rhs=wsrc[:],
                    start=True,
                    stop=True,
                )

            for pair in range(PAIRS):
                xt = xp.tile([128, INCOLS], mybir.dt.bfloat16)
                if pair == 0:
                    # Split the first load so image-half 0 (+ weights) lands
                    # sooner and the real matmuls start earlier.
                    nc.sync.dma_start(
                        out=xt[:, 0 : NHALF * W], in_=xd[pair, :, 0 : NHALF * W]
                    )
                    nc.sync.dma_start(
                        out=xt[:, XCOLS:INCOLS], in_=xd[pair, :, XCOLS:INCOLS]
                    )
                    nc.sync.dma_start(
                        out=xt[:, NHALF * W : XCOLS],
                        in_=xd[pair, :, NHALF * W : XCOLS],
                    )
                else:
                    nc.sync.dma_start(out=xt[:], in_=xd[pair])
                xv = xt[:, 0:XCOLS].rearrange("p (n w) -> p n w", w=W)
                wv = xt[:, XCOLS:INCOLS].rearrange("p (k m) -> p k m", m=H)

                pts = [
                    ps.tile([128, NHALF, WO], mybir.dt.float32, name=f"pt{h}")
                    for h in range(2)
                ]
                for kw, xc0, xc1, wo0, wo1 in KW_PLAN:
                    for half in range(2):
                        rhs = xv[:, half * NHALF : (half + 1) * NHALF, xc0:xc1]
                        # Channel A in PE quadrants (rows 0-63, cols 0-63),
                        # channel B in (rows 64-127, cols 64-127): the two
                        # matmuls run concurrently in disjoint subarrays.
                        # NOTE: kw-outer order is load-bearing — consecutive
                        # slots share lhsT, which is what lets the PE stream
                        # at ~215 ns/slot (half-serial order measured 86 us).
                        nc.tensor.matmul(
                            pts[half][0:64, :, wo0:wo1],
                            lhsT=wv[0:64, kw, :],
                            rhs=rhs[0:64],
                            start=(kw == 1),
                            stop=(kw == 3),
                            tile_position=(0, 0),
                        )
                        nc.tensor.matmul(
                            pts[half][64:128, :, wo0:wo1],
                            lhsT=wv[64:128, kw, :],
                            rhs=rhs[64:128],
                            start=(kw == 1),
                            stop=(kw == 3),
                            tile_position=(64, 64),
                        )
                ot = op.tile([128, 2, NFREE], mybir.dt.bfloat16)
                if pair == PAIRS - 1:
                    # Tail: split each copy across ACT and DVE and store each
                    # half as soon as its copy lands.
                    for half in range(2):
                        nc.scalar.copy(
                            ot[:, half, 0 : NFREE // 2], pts[half][:, 0:4, :]
                        )
                        nc.vector.tensor_copy(
                            ot[:, half, NFREE // 2 : NFREE], pts[half][:, 4:8, :]
                        )
                        # Sync's HWDGE ring is idle by the tail (inputs done);
                        # dispatch the two half-stores on different rings so
                        # they overlap instead of serializing on Scalar.
                        eng = nc.scalar if half == 0 else nc.sync
                        eng.dma_start(out=od[pair, :, half], in_=ot[:, half, :])
                else:
                    nc.scalar.copy(ot[:, 0, :], pts[0][:])
                    nc.vector.tensor_copy(ot[:, 1, :], pts[1][:])
                    nc.scalar.dma_start(out=od[pair], in_=ot[:])
    nc.compile()
    return nc


def _get_nc():
    global _NC_CACHE
    if _NC_CACHE is None:
        _NC_CACHE = _build_nc()
    return _NC_CACHE


def _prep_x(x):
    """x (16, 512, 64, 64) f32 -> per-core (PAIRS, 128, XCOLS) bf16.

    Partition index p = c'*64 + h for channel pair slot c' in {0, 1};
    free layout [n, w] (no pad columns - kw edges use clipped matmul ranges).
    """
    maps = []
    for k in range(N_CORES):
        xc = x[:, k * CH : (k + 1) * CH]  # (16, 64, 64, 64)
        t = xc.transpose(1, 2, 0, 3)  # (ch, h, n, w)
        maps.append(t.astype(BF16).reshape(PAIRS, 128, XCOLS))
    return maps


def _prep_w(wc):
    """wc (512, 4, 4) f32 masked per-channel weights ->
    per-core (PAIRS, 128, WCOLS) bf16 banded lhsT blocks.

    lhsT[pair, c'*64 + h, kw*H + ho] = wc[ch, h - ho + 1, kw]
    for 0 <= h - ho + 1 <= 3, ho <= 62 (column 63 stays zero).
    """
    maps = []
    for k in range(N_CORES):
        wk = wc[k * CH : (k + 1) * CH]  # (64, 4, 4) [ch, kh, kw]
        blocks = np.zeros((CH, 4, H, H), dtype=np.float32)  # [ch, kw, h, ho]
        ho = np.arange(HO)
        for kh in range(4):
            h = ho + kh - 1
            v = (h >= 0) & (h < H)
            blocks[:, :, h[v], ho[v]] = wk[:, kh, :][:, :, None]
        # [ch, kw, h, ho] -> [pair, c'*64+h, kw*H + ho]
        lt = blocks.transpose(0, 2, 1, 3).reshape(PAIRS, 128, WCOLS)
        maps.append(lt.astype(BF16))
    return maps


def _prep_in(x, wc):
    xs = _prep_x(x)
    ws = _prep_w(wc)
    return [
        np.ascontiguousarray(np.concatenate([xs[k], ws[k]], axis=2))
        for k in range(N_CORES)
    ]


def _unswizzle(out_dev):
    """(PAIRS, 128, 2, NFREE) bf16 -> (16, 64, 63, 63) f32 for one core."""
    r = out_dev.reshape(PAIRS, 2, H, 2, NHALF, WO)  # [pair, c', ho64, half, n', wo]
    t = r.transpose(3, 4, 0, 1, 2, 5)  # [half, n', pair, c', ho64, wo]
    return np.ascontiguousarray(
        t.reshape(IMGS, CH, H, WO)[:, :, :HO, :].astype(np.float32)
    )


def kernel(x, weight, mask, groups=8, stride=1, _trace=False, _trace_kwargs=None):
    global LAST_RESULT
    x = np.ascontiguousarray(np.asarray(x, dtype=np.float32))
    weight = np.asarray(weight, dtype=np.float32)
    mask = np.asarray(mask, dtype=np.float32)

    # Masked weights collapse to one 4x4 filter per output channel.
    wc = (weight * mask).sum(axis=1)  # (512, 4, 4)

    ins = _prep_in(x, wc)
    in_maps = [{"xin": ins[k]} for k in range(N_CORES)]

    nc = _get_nc()
    kwargs = {}
    if _trace:
        kwargs["trace"] = True
        if _trace_kwargs:
            kwargs.update(_trace_kwargs)
    res = run_bass_kernel_spmd(nc, in_maps, core_ids=list(range(N_CORES)), **kwargs)
    LAST_RESULT = res

    outs = [_unswizzle(res.results[k]["out"]) for k in range(N_CORES)]
    return np.concatenate(outs, axis=1)


def emulate(x, weight, mask, groups=8, stride=1):
    """Pure-numpy emulation of the device math (same bf16 rounding and
    packing) - validates host prep + Toeplitz construction without HW."""
    x = np.asarray(x, dtype=np.float32)
    wc = (np.asarray(weight, np.float32) * np.asarray(mask, np.float32)).sum(axis=1)
    ins = _prep_in(x, wc)
    outs = []
    for k in range(N_CORES):
        out_dev = np.zeros((PAIRS, 128, 2, NFREE), dtype=BF16)
        for pair in range(PAIRS):
            xin = ins[k][pair, :, 0:XCOLS].astype(np.float32)
            xin = xin.reshape(128, IMGS, W)
            wt = ins[k][pair, :, XCOLS:INCOLS].astype(np.float32)
            wt = wt.reshape(128, 4, H)
            for half in range(2):
                acc = np.zeros((128, NHALF, WO), dtype=np.float32)
                for kw, xc0, xc1, wo0, wo1 in KW_PLAN:
                    rhs = xin[:, half * NHALF : (half + 1) * NHALF, xc0:xc1]
                    acc[0:64, :, wo0:wo1] += np.einsum(
                        'km,knw->mnw', wt[0:64, kw, :], rhs[0:64])
                    acc[64:128, :, wo0:wo1] += np.einsum(
                        'km,knw->mnw', wt[64:128, kw, :], rhs[64:128])
                out_dev[pair, :, half, :] = acc.reshape(128, NFREE).astype(BF16)
        outs.append(_unswizzle(out_dev))
    return np.concatenate(outs, axis=1)


# revision 31
# speedup vs baseline: 1.0629x; 1.0020x over previous
# Depthwise 4x4 conv (DiagonalwiseRefactorization) on 8 TRN2 NeuronCores.
#
# The mask zeroes every weight except weight[c, c % 64], and with
# feature_group_count=8 the grouped conv collapses to a depthwise conv:
#   out[n, c, ho, wo] = sum_{kh, kw} w[c, kh, kw] * xpad[n, c, ho+kh, wo+kw]
# with pad=1, stride=1: (16, 512, 64, 64) -> (16, 512, 63, 63).
#
# Device strategy (per core: 64 channels x 16 images, no inter-core comm):
#   For each width-tap kw, the H-direction conv is a banded-Toeplitz matmul:
#     out[c, :, wo] += T_c_kw.T @ xrow[c, :, wo+kw]
#   where T_c_kw[h, ho] = w[c, h-ho+1, kw] (64x64, 4 diagonals; the H padding
#   falls out of the band clipping). Two channels run concurrently in opposite
#   32x32-quadrant groups of the PE array (tile_position (0,0) / (64,64));
#   the 4 kw taps accumulate in PSUM. bf16 matmuls, fp32 PSUM accumulate,
#   bf16 store (un-swizzled + upcast on host).
#
# Host does layout only: bf16 cast + swizzle of x to [pair, c'*64+h, n, wpad],
# building the banded lhsT blocks from (weight*mask), and un-permuting the
# swizzled output.

import sys
import types

import numpy as np
import ml_dtypes

BF16 = ml_dtypes.bfloat16

N_CORES = 8
IMGS = 16
CH_TOT = 512
CH = CH_TOT // N_CORES  # 64 channels per core
PAIRS = CH // 2  # 32
H = W = 64
HO = WO = 63
WPAD = W + 2  # 66 (one zero column each side)
NHALF = IMGS // 2  # 8 images per psum tile
NFREE = NHALF * WO  # 504 <= 512 (one PSUM bank)


def _install_axon_hooks_shim():
    """Make trace=True work under axon: bass_utils imports
    antenv.axon_hooks, which the container's antenv stub lacks."""
    try:
        import antenv.axon_hooks  # noqa: F401

        return
    except ImportError:
        pass
    try:
        import antenv
    except ImportError:
        return
    mod = types.ModuleType("antenv.axon_hooks")
    mod._hook = None

    def set_axon_ntff_profile_hook(h):
        mod._hook = h

    def get_axon_ntff_profile_hook():
        return mod._hook

    mod.set_axon_ntff_profile_hook = set_axon_ntff_profile_hook
    mod.get_axon_ntff_profile_hook = get_axon_ntff_profile_hook
    sys.modules["antenv.axon_hooks"] = mod
    antenv.axon_hooks = mod
    try:
        from trn_agent_boot.trn_boot import _ntff_profile_via_ctypes

        hook = _ntff_profile_via_ctypes("/opt/axon/libaxon_pjrt.so")
        if hook is not None:
            mod._hook = hook
    except Exception:
        pass


_install_axon_hooks_shim()

import concourse.bacc as bacc  # noqa: E402
import concourse.mybir as mybir  # noqa: E402
import concourse.tile as tile  # noqa: E402
from concourse.bass_utils import run_bass_kernel_spmd  # noqa: E402

LAST_RESULT = None
_NC_CACHE = None


XCOLS = IMGS * W  # 1024 (no pad columns; kw edges handled by clipped matmuls)
WCOLS = 4 * H  # 256
INCOLS = XCOLS + WCOLS  # 1280: x and lhsT packed into one DMA per pair

# Per width-tap kw: x column range [xc0, xc1) and output wo range [wo0, wo1).
# out[wo] += w[.., kw] * x[wo + kw - 1]; clipped where x would be padding.
# kw=1 goes first: it covers the full wo range, so its start=True write sets
# PSUM has_written everywhere before the partial-range taps accumulate.
KW_PLAN = [
    (1, 0, 63, 0, 63),  # kw, xc0, xc1, wo0, wo1
    (2, 1, 64, 0, 63),
    (0, 0, 62, 1, 63),
    (3, 2, 64, 0, 62),
]


def _build_nc():
    # Bass.__init__ emits four [128,1] const-AP memsets on GpSimd whose DMA
    # completion delays the first all-engine barrier; this kernel never reads
    # the const APs (matmul/copy/dma only), so skip those preamble memsets.
    import concourse.bass as bassmod

    orig_memset = bassmod.BassGpSimd.memset
    bassmod.BassGpSimd.memset = lambda self, ap, constant: None
    try:
        nc = bacc.Bacc(
            "TRN2", target_bir_lowering=False, debug=False, num_devices=N_CORES
        )
    finally:
        bassmod.BassGpSimd.memset = orig_memset

    xd = nc.dram_tensor(
        "xin", [PAIRS, 128, INCOLS], mybir.dt.bfloat16, kind="ExternalInput"
    )
    od = nc.dram_tensor(
        "out", [PAIRS, 128, 2, NFREE], mybir.dt.bfloat16, kind="ExternalOutput"
    )

    with tile.TileContext(nc) as tc:
        with (
            tc.tile_pool(name="xp", bufs=8) as xp,
            tc.tile_pool(name="ps", bufs=4, space="PSUM") as ps,
            tc.tile_pool(name="op", bufs=8) as op,
        ):
            # Warm up the PE HAM clock gate (1.2 -> 2.4 GHz needs ~3.4 us of
            # sustained matmul activity) inside the first x-DMA's shadow, so
            # the real matmuls start at full clock.
            wsrc = op.tile([128, 128], mybir.dt.bfloat16, name="warmsrc")
            nc.vector.memset(wsrc[:], 0.0)
            warm = ps.tile([128, NFREE], mybir.dt.float32, name="pt0")
            for _ in range(27):
                nc.tensor.matmul(
                    warm[0:64, 0:128],
                    lhsT=wsrc[:, 0:64],
                    rhs=wsrc[:],
                    start=True,
                    stop=True,
                )

            # Pair 0 split so image-half 0 (+ weights) lands sooner; pair 1's
            # load dispatches BEFORE pair 0's second-half chunk (needed ~0.9us
            # later than pair 1... pair 0's early matmuls only gate on the
            # first two chunks - Tile tracks writes per region).
            early = {}
            for pair in (0, 1):
                early[pair] = xp.tile([128, INCOLS], mybir.dt.bfloat16, name="xt")
            nc.sync.dma_start(out=early[0][:, 0 : NHALF * W], in_=xd[0, :, 0 : NHALF * W])
            nc.sync.dma_start(out=early[0][:, XCOLS:INCOLS], in_=xd[0, :, XCOLS:INCOLS])
            nc.sync.dma_start(out=early[1][:], in_=xd[1])
            nc.sync.dma_start(
                out=early[0][:, NHALF * W : XCOLS], in_=xd[0, :, NHALF * W : XCOLS]
            )
            for pair in range(PAIRS):
                if pair in early:
                    xt = early[pair]
                else:
                    xt = xp.tile([128, INCOLS], mybir.dt.bfloat16, name="xt")
                    nc.sync.dma_start(out=xt[:], in_=xd[pair])
                xv = xt[:, 0:XCOLS].rearrange("p (n w) -> p n w", w=W)
                wv = xt[:, XCOLS:INCOLS].rearrange("p (k m) -> p k m", m=H)

                pts = [
                    ps.tile([128, NHALF, WO], mybir.dt.float32, name=f"pt{h}")
                    for h in range(2)
                ]
                for kw, xc0, xc1, wo0, wo1 in KW_PLAN:
                    for half in range(2):
                        rhs = xv[:, half * NHALF : (half + 1) * NHALF, xc0:xc1]
                        # Channel A in PE quadrants (rows 0-63, cols 0-63),
                        # channel B in (rows 64-127, cols 64-127): the two
                        # matmuls run concurrently in disjoint subarrays.
                        # NOTE: kw-outer order is load-bearing — consecutive
                        # slots share lhsT, which is what lets the PE stream
                        # at ~215 ns/slot (half-serial order measured 86 us).
                        nc.tensor.matmul(
                            pts[half][0:64, :, wo0:wo1],
                            lhsT=wv[0:64, kw, :],
                            rhs=rhs[0:64],
                            start=(kw == 1),
                            stop=(kw == 3),
                            tile_position=(0, 0),
                        )
                        nc.tensor.matmul(
                            pts[half][64:128, :, wo0:wo1],
                            lhsT=wv[64:128, kw, :],
                            rhs=rhs[64:128],
                            start=(kw == 1),
                            stop=(kw == 3),
                            tile_position=(64, 64),
                        )
                ot = op.tile([128, 2, NFREE], mybir.dt.bfloat16)
                if pair == PAIRS - 1:
                    # Tail: split each copy across ACT and DVE and store each
                    # half as soon as its copy lands.
                    for half in range(2):
                        nc.scalar.copy(
                            ot[:, half, 0 : NFREE // 2], pts[half][:, 0:4, :]
                        )
                        nc.vector.tensor_copy(
                            ot[:, half, NFREE // 2 : NFREE], pts[half][:, 4:8, :]
                        )
                        # Sync's HWDGE ring is idle by the tail (inputs done);
                        # dispatch the two half-stores on different rings so
                        # they overlap instead of serializing on Scalar.
                        eng = nc.scalar if half == 0 else nc.sync
                        eng.dma_start(out=od[pair, :, half], in_=ot[:, half, :])
                else:
                    nc.scalar.copy(ot[:, 0, :], pts[0][:])
                    nc.vector.tensor_copy(ot[:, 1, :], pts[1][:])
                    nc.scalar.dma_start(out=od[pair], in_=ot[:])
    nc.compile()
    return nc


def _get_nc():
    global _NC_CACHE
    if _NC_CACHE is None:
        _NC_CACHE = _build_nc()
    return _NC_CACHE


def _prep_x(x):
    """x (16, 512, 64, 64) f32 -> per-core (PAIRS, 128, XCOLS) bf16.

    Partition index p = c'*64 + h for channel pair slot c' in {0, 1};
    free layout [n, w] (no pad columns - kw edges use clipped matmul ranges).
    """
    maps = []
    for k in range(N_CORES):
        xc = x[:, k * CH : (k + 1) * CH]  # (16, 64, 64, 64)
        t = xc.transpose(1, 2, 0, 3)  # (ch, h, n, w)
        maps.append(t.astype(BF16).reshape(PAIRS, 128, XCOLS))
    return maps


def _prep_w(wc):
    """wc (512, 4, 4) f32 masked per-channel weights ->
    per-core (PAIRS, 128, WCOLS) bf16 banded lhsT blocks.

    lhsT[pair, c'*64 + h, kw*H + ho] = wc[ch, h - ho + 1, kw]
    for 0 <= h - ho + 1 <= 3, ho <= 62 (column 63 stays zero).
    """
    maps = []
    for k in range(N_CORES):
        wk = wc[k * CH : (k + 1) * CH]  # (64, 4, 4) [ch, kh, kw]
        blocks = np.zeros((CH, 4, H, H), dtype=np.float32)  # [ch, kw, h, ho]
        ho = np.arange(HO)
        for kh in range(4):
            h = ho + kh - 1
            v = (h >= 0) & (h < H)
            blocks[:, :, h[v], ho[v]] = wk[:, kh, :][:, :, None]
        # [ch, kw, h, ho] -> [pair, c'*64+h, kw*H + ho]
        lt = blocks.transpose(0, 2, 1, 3).reshape(PAIRS, 128, WCOLS)
        maps.append(lt.astype(BF16))
    return maps


def _prep_in(x, wc):
    xs = _prep_x(x)
    ws = _prep_w(wc)
    return [
        np.ascontiguousarray(np.concatenate([xs[k], ws[k]], axis=2))
        for k in range(N_CORES)
    ]


def _unswizzle(out_dev):
    """(PAIRS, 128, 2, NFREE) bf16 -> (16, 64, 63, 63) f32 for one core."""
    r = out_dev.reshape(PAIRS, 2, H, 2, NHALF, WO)  # [pair, c', ho64, half, n', wo]
    t = r.transpose(3, 4, 0, 1, 2, 5)  # [half, n', pair, c', ho64, wo]
    return np.ascontiguousarray(
        t.reshape(IMGS, CH, H, WO)[:, :, :HO, :].astype(np.float32)
    )


def kernel(x, weight, mask, groups=8, stride=1, _trace=False, _trace_kwargs=None):
    global LAST_RESULT
    x = np.ascontiguousarray(np.asarray(x, dtype=np.float32))
    weight = np.asarray(weight, dtype=np.float32)
    mask = np.asarray(mask, dtype=np.float32)

    # Masked weights collapse to one 4x4 filter per output channel.
    wc = (weight * mask).sum(axis=1)  # (512, 4, 4)

    ins = _prep_in(x, wc)
    in_maps = [{"xin": ins[k]} for k in range(N_CORES)]

    nc = _get_nc()
    kwargs = {}
    if _trace:
        kwargs["trace"] = True
        if _trace_kwargs:
            kwargs.update(_trace_kwargs)
    res = run_bass_kernel_spmd(nc, in_maps, core_ids=list(range(N_CORES)), **kwargs)
    LAST_RESULT = res

    outs = [_unswizzle(res.results[k]["out"]) for k in range(N_CORES)]
    return np.concatenate(outs, axis=1)


def emulate(x, weight, mask, groups=8, stride=1):
    """Pure-numpy emulation of the device math (same bf16 rounding and
    packing) - validates host prep + Toeplitz construction without HW."""
    x = np.asarray(x, dtype=np.float32)
    wc = (np.asarray(weight, np.float32) * np.asarray(mask, np.float32)).sum(axis=1)
    ins = _prep_in(x, wc)
    outs = []
    for k in range(N_CORES):
        out_dev = np.zeros((PAIRS, 128, 2, NFREE), dtype=BF16)
        for pair in range(PAIRS):
            xin = ins[k][pair, :, 0:XCOLS].astype(np.float32)
            xin = xin.reshape(128, IMGS, W)
            wt = ins[k][pair, :, XCOLS:INCOLS].astype(np.float32)
            wt = wt.reshape(128, 4, H)
            for half in range(2):
                acc = np.zeros((128, NHALF, WO), dtype=np.float32)
                for kw, xc0, xc1, wo0, wo1 in KW_PLAN:
                    rhs = xin[:, half * NHALF : (half + 1) * NHALF, xc0:xc1]
                    acc[0:64, :, wo0:wo1] += np.einsum(
                        'km,knw->mnw', wt[0:64, kw, :], rhs[0:64])
                    acc[64:128, :, wo0:wo1] += np.einsum(
                        'km,knw->mnw', wt[64:128, kw, :], rhs[64:128])
                out_dev[pair, :, half, :] = acc.reshape(128, NFREE).astype(BF16)
        outs.append(_unswizzle(out_dev))
    return np.concatenate(outs, axis=1)


# revision 32
# speedup vs baseline: 1.0649x; 1.0019x over previous
# Depthwise 4x4 conv (DiagonalwiseRefactorization) on 8 TRN2 NeuronCores.
#
# The mask zeroes every weight except weight[c, c % 64], and with
# feature_group_count=8 the grouped conv collapses to a depthwise conv:
#   out[n, c, ho, wo] = sum_{kh, kw} w[c, kh, kw] * xpad[n, c, ho+kh, wo+kw]
# with pad=1, stride=1: (16, 512, 64, 64) -> (16, 512, 63, 63).
#
# Device strategy (per core: 64 channels x 16 images, no inter-core comm):
#   For each width-tap kw, the H-direction conv is a banded-Toeplitz matmul:
#     out[c, :, wo] += T_c_kw.T @ xrow[c, :, wo+kw]
#   where T_c_kw[h, ho] = w[c, h-ho+1, kw] (64x64, 4 diagonals; the H padding
#   falls out of the band clipping). Two channels run concurrently in opposite
#   32x32-quadrant groups of the PE array (tile_position (0,0) / (64,64));
#   the 4 kw taps accumulate in PSUM. bf16 matmuls, fp32 PSUM accumulate,
#   bf16 store (un-swizzled + upcast on host).
#
# Host does layout only: bf16 cast + swizzle of x to [pair, c'*64+h, n, wpad],
# building the banded lhsT blocks from (weight*mask), and un-permuting the
# swizzled output.

import sys
import types

import numpy as np
import ml_dtypes

BF16 = ml_dtypes.bfloat16

N_CORES = 8
IMGS = 16
CH_TOT = 512
CH = CH_TOT // N_CORES  # 64 channels per core
PAIRS = CH // 2  # 32
H = W = 64
HO = WO = 63
WPAD = W + 2  # 66 (one zero column each side)
NHALF = IMGS // 2  # 8 images per psum tile
NFREE = NHALF * WO  # 504 <= 512 (one PSUM bank)


def _install_axon_hooks_shim():
    """Make trace=True work under axon: bass_utils imports
    antenv.axon_hooks, which the container's antenv stub lacks."""
    try:
        import antenv.axon_hooks  # noqa: F401

        return
    except ImportError:
        pass
    try:
        import antenv
    except ImportError:
        return
    mod = types.ModuleType("antenv.axon_hooks")
    mod._hook = None

    def set_axon_ntff_profile_hook(h):
        mod._hook = h

    def get_axon_ntff_profile_hook():
        return mod._hook

    mod.set_axon_ntff_profile_hook = set_axon_ntff_profile_hook
    mod.get_axon_ntff_profile_hook = get_axon_ntff_profile_hook
    sys.modules["antenv.axon_hooks"] = mod
    antenv.axon_hooks = mod
    try:
        from trn_agent_boot.trn_boot import _ntff_profile_via_ctypes

        hook = _ntff_profile_via_ctypes("/opt/axon/libaxon_pjrt.so")
        if hook is not None:
            mod._hook = hook
    except Exception:
        pass


_install_axon_hooks_shim()

import concourse.bacc as bacc  # noqa: E402
import concourse.mybir as mybir  # noqa: E402
import concourse.tile as tile  # noqa: E402
from concourse.bass_utils import run_bass_kernel_spmd  # noqa: E402

LAST_RESULT = None
_NC_CACHE = None


XCOLS = IMGS * W  # 1024 (no pad columns; kw edges handled by clipped matmuls)
WCOLS = 4 * H  # 256
INCOLS = XCOLS + WCOLS  # 1280: x and lhsT packed into one DMA per pair

# Per width-tap kw: x column range [xc0, xc1) and output wo range [wo0, wo1).
# out[wo] += w[.., kw] * x[wo + kw - 1]; clipped where x would be padding.
# kw=1 goes first: it covers the full wo range, so its start=True write sets
# PSUM has_written everywhere before the partial-range taps accumulate.
KW_PLAN = [
    (1, 0, 63, 0, 63),  # kw, xc0, xc1, wo0, wo1
    (2, 1, 64, 0, 63),
    (0, 0, 62, 1, 63),
    (3, 2, 64, 0, 62),
]


def _build_nc():
    # Bass.__init__ emits four [128,1] const-AP memsets on GpSimd whose DMA
    # completion delays the first all-engine barrier; this kernel never reads
    # the const APs (matmul/copy/dma only), so skip those preamble memsets.
    import concourse.bass as bassmod

    orig_memset = bassmod.BassGpSimd.memset
    bassmod.BassGpSimd.memset = lambda self, ap, constant: None
    try:
        nc = bacc.Bacc(
            "TRN2", target_bir_lowering=False, debug=False, num_devices=N_CORES
        )
    finally:
        bassmod.BassGpSimd.memset = orig_memset

    xd = nc.dram_tensor(
        "xin", [PAIRS, 128, INCOLS], mybir.dt.bfloat16, kind="ExternalInput"
    )
    od = nc.dram_tensor(
        "out", [PAIRS, 128, 2, NFREE], mybir.dt.bfloat16, kind="ExternalOutput"
    )

    with tile.TileContext(nc) as tc:
        with (
            tc.tile_pool(name="xp", bufs=8) as xp,
            tc.tile_pool(name="ps", bufs=4, space="PSUM") as ps,
            tc.tile_pool(name="op", bufs=8) as op,
        ):
            # Warm up the PE HAM clock gate (1.2 -> 2.4 GHz needs ~3.4 us of
            # sustained matmul activity) inside the first x-DMA's shadow, so
            # the real matmuls start at full clock.
            wsrc = op.tile([128, 128], mybir.dt.bfloat16, name="warmsrc")
            nc.vector.memset(wsrc[:], 0.0)
            warm = ps.tile([128, NFREE], mybir.dt.float32, name="pt0")
            for _ in range(27):
                nc.tensor.matmul(
                    warm[0:64, 0:128],
                    lhsT=wsrc[:, 0:64],
                    rhs=wsrc[:],
                    start=True,
                    stop=True,
                )

            for pair in range(PAIRS):
                xt = xp.tile([128, INCOLS], mybir.dt.bfloat16)
                if pair == 0:
                    # Split the first load so image-half 0 (+ weights) lands
                    # sooner and the real matmuls start earlier.
                    nc.sync.dma_start(
                        out=xt[:, 0 : NHALF * W], in_=xd[pair, :, 0 : NHALF * W]
                    )
                    nc.sync.dma_start(
                        out=xt[:, XCOLS:INCOLS], in_=xd[pair, :, XCOLS:INCOLS]
                    )
                    nc.sync.dma_start(
                        out=xt[:, NHALF * W : XCOLS],
                        in_=xd[pair, :, NHALF * W : XCOLS],
                    )
                else:
                    nc.sync.dma_start(out=xt[:], in_=xd[pair])
                xv = xt[:, 0:XCOLS].rearrange("p (n w) -> p n w", w=W)
                wv = xt[:, XCOLS:INCOLS].rearrange("p (k m) -> p k m", m=H)

                pts = [
                    ps.tile([128, NHALF, WO], mybir.dt.float32, name=f"pt{h}")
                    for h in range(2)
                ]
                for kw, xc0, xc1, wo0, wo1 in KW_PLAN:
                    for half in range(2):
                        rhs = xv[:, half * NHALF : (half + 1) * NHALF, xc0:xc1]
                        # Channel A in PE quadrants (rows 0-63, cols 0-63),
                        # channel B in (rows 64-127, cols 64-127): the two
                        # matmuls run concurrently in disjoint subarrays.
                        # NOTE: kw-outer order is load-bearing — consecutive
                        # slots share lhsT, which is what lets the PE stream
                        # at ~215 ns/slot (half-serial order measured 86 us).
                        nc.tensor.matmul(
                            pts[half][0:64, :, wo0:wo1],
                            lhsT=wv[0:64, kw, :],
                            rhs=rhs[0:64],
                            start=(kw == 1),
                            stop=(kw == 3),
                            tile_position=(0, 0),
                        )
                        nc.tensor.matmul(
                            pts[half][64:128, :, wo0:wo1],
                            lhsT=wv[64:128, kw, :],
                            rhs=rhs[64:128],
                            start=(kw == 1),
                            stop=(kw == 3),
                            tile_position=(64, 64),
                        )
                ot = op.tile([128, 2, NFREE], mybir.dt.bfloat16)
                if pair == PAIRS - 1:
                    # Tail: split each copy across ACT and DVE and store each
                    # half as soon as its copy lands.
                    for half in range(2):
                        nc.scalar.copy(
                            ot[:, half, 0 : NFREE // 2], pts[half][:, 0:4, :]
                        )
                        nc.vector.tensor_copy(
                            ot[:, half, NFREE // 2 : NFREE], pts[half][:, 4:8, :]
                        )
                        # Sync's HWDGE ring is idle by the tail (inputs done);
                        # dispatch the two half-stores on different rings so
                        # they overlap instead of serializing on Scalar.
                        eng = nc.scalar if half == 0 else nc.sync
                        eng.dma_start(out=od[pair, :, half], in_=ot[:, half, :])
                else:
                    nc.scalar.copy(ot[:, 0, :], pts[0][:])
                    nc.vector.tensor_copy(ot[:, 1, :], pts[1][:])
                    nc.scalar.dma_start(out=od[pair], in_=ot[:])
    nc.compile()
    return nc


def _get_nc():
    global _NC_CACHE
    if _NC_CACHE is None:
        _NC_CACHE = _build_nc()
    return _NC_CACHE


def _prep_x(x):
    """x (16, 512, 64, 64) f32 -> per-core (PAIRS, 128, XCOLS) bf16.

    Partition index p = c'*64 + h for channel pair slot c' in {0, 1};
    free layout [n, w] (no pad columns - kw edges use clipped matmul ranges).
    """
    maps = []
    for k in range(N_CORES):
        xc = x[:, k * CH : (k + 1) * CH]  # (16, 64, 64, 64)
        t = xc.transpose(1, 2, 0, 3)  # (ch, h, n, w)
        maps.append(t.astype(BF16).reshape(PAIRS, 128, XCOLS))
    return maps


def _prep_w(wc):
    """wc (512, 4, 4) f32 masked per-channel weights ->
    per-core (PAIRS, 128, WCOLS) bf16 banded lhsT blocks.

    lhsT[pair, c'*64 + h, kw*H + ho] = wc[ch, h - ho + 1, kw]
    for 0 <= h - ho + 1 <= 3, ho <= 62 (column 63 stays zero).
    """
    maps = []
    for k in range(N_CORES):
        wk = wc[k * CH : (k + 1) * CH]  # (64, 4, 4) [ch, kh, kw]
        blocks = np.zeros((CH, 4, H, H), dtype=np.float32)  # [ch, kw, h, ho]
        ho = np.arange(HO)
        for kh in range(4):
            h = ho + kh - 1
            v = (h >= 0) & (h < H)
            blocks[:, :, h[v], ho[v]] = wk[:, kh, :][:, :, None]
        # [ch, kw, h, ho] -> [pair, c'*64+h, kw*H + ho]
        lt = blocks.transpose(0, 2, 1, 3).reshape(PAIRS, 128, WCOLS)
        maps.append(lt.astype(BF16))
    return maps


def _prep_in(x, wc):
    xs = _prep_x(x)
    ws = _prep_w(wc)
    return [
        np.ascontiguousarray(np.concatenate([xs[k], ws[k]], axis=2))
        for k in range(N_CORES)
    ]


def _unswizzle(out_dev):
    """(PAIRS, 128, 2, NFREE) bf16 -> (16, 64, 63, 63) f32 for one core."""
    r = out_dev.reshape(PAIRS, 2, H, 2, NHALF, WO)  # [pair, c', ho64, half, n', wo]
    t = r.transpose(3, 4, 0, 1, 2, 5)  # [half, n', pair, c', ho64, wo]
    return np.ascontiguousarray(
        t.reshape(IMGS, CH, H, WO)[:, :, :HO, :].astype(np.float32)
    )


def kernel(x, weight, mask, groups=8, stride=1, _trace=False, _trace_kwargs=None):
    global LAST_RESULT
    x = np.ascontiguousarray(np.asarray(x, dtype=np.float32))
    weight = np.asarray(weight, dtype=np.float32)
    mask = np.asarray(mask, dtype=np.float32)

    # Masked weights collapse to one 4x4 filter per output channel.
    wc = (weight * mask).sum(axis=1)  # (512, 4, 4)

    ins = _prep_in(x, wc)
    in_maps = [{"xin": ins[k]} for k in range(N_CORES)]

    nc = _get_nc()
    kwargs = {}
    if _trace:
        kwargs["trace"] = True
        if _trace_kwargs:
            kwargs.update(_trace_kwargs)
    res = run_bass_kernel_spmd(nc, in_maps, core_ids=list(range(N_CORES)), **kwargs)
    LAST_RESULT = res

    outs = [_unswizzle(res.results[k]["out"]) for k in range(N_CORES)]
    return np.concatenate(outs, axis=1)


def emulate(x, weight, mask, groups=8, stride=1):
    """Pure-numpy emulation of the device math (same bf16 rounding and
    packing) - validates host prep + Toeplitz construction without HW."""
    x = np.asarray(x, dtype=np.float32)
    wc = (np.asarray(weight, np.float32) * np.asarray(mask, np.float32)).sum(axis=1)
    ins = _prep_in(x, wc)
    outs = []
    for k in range(N_CORES):
        out_dev = np.zeros((PAIRS, 128, 2, NFREE), dtype=BF16)
        for pair in range(PAIRS):
            xin = ins[k][pair, :, 0:XCOLS].astype(np.float32)
            xin = xin.reshape(128, IMGS, W)
            wt = ins[k][pair, :, XCOLS:INCOLS].astype(np.float32)
            wt = wt.reshape(128, 4, H)
            for half in range(2):
                acc = np.zeros((128, NHALF, WO), dtype=np.float32)
                for kw, xc0, xc1, wo0, wo1 in KW_PLAN:
                    rhs = xin[:, half * NHALF : (half + 1) * NHALF, xc0:xc1]
                    acc[0:64, :, wo0:wo1] += np.einsum(
                        'km,knw->mnw', wt[0:64, kw, :], rhs[0:64])
                    acc[64:128, :, wo0:wo1] += np.einsum(
                        'km,knw->mnw', wt[64:128, kw, :], rhs[64:128])
                out_dev[pair, :, half, :] = acc.reshape(128, NFREE).astype(BF16)
        outs.append(_unswizzle(out_dev))
    return np.concatenate(outs, axis=1)
